# revision 20
# baseline (speedup 1.0000x reference)
"""TensorProductConvLayer (DiffDock) Bass kernel for 8 Trainium2 cores.

The wall clock is dominated by the axon tunnel (~50-80 MB/s shared), so the
design minimizes wire bytes in both directions:
  - Edges are pre-sorted by source node on the host; each core gets a
    contiguous shard of 125K sorted edges.
  - edge_attr ships fp8 e3m4 (x2 scale, un-done by the activation scale),
    x = node_attr[dst] ships fp16 non-replicated, sh coefficients and
    block-local node ids ship as a small fp16 sideband tensor, and all
    model constants ship as one packed fp16 tensor (one device_put).
  - The device computes the per-edge MLP + tensor product AND the segment
    sum: per 500-edge block it emits [64 local nodes, 28] partial sums
    (one-hot matmul against iota-built masks), so D2H is 7 MB instead of
    per-edge outputs (40+ MB). The host overlap-adds block partials and
    divides by counts.
Per block: edge-major fp8 rows transposed on the PE (f16 identity matmul),
MLP on PE (fp16 weights), TP contraction as DVE elementwise multiply + PE
reduction emitting edge-major poT [125, 20] (operand-swapped matmul), DVE
per-partition sh scaling, is_equal one-hot build, segment matmul. Host prep
(fp16 cast -> e3m4 LUT -> sort-order gather) runs as one fused jax-CPU jit;
the serialized BIR is cached on the nc object so warm calls skip
re-serialization.
"""

import os
import tempfile
import numpy as np
import ml_dtypes

E_TOT = 1_000_000
N_NODES = 100_000
NCORES = 8
ESH = E_TOT // NCORES          # 125000 edges per core
BLK = 500
NB = ESH // BLK                # 250 blocks, no padding
CHK = 125                      # edges per transposed reduction chunk
OH = 64                        # one-hot width (max block node span, asserted)
EA_SCALE = np.float32(2.0)     # edge_attr pre-scale for fp8 e3m4 range use

F8 = ml_dtypes.float8_e3m4

_CACHE = {}
LAST_RESULTS = None


def _build_bass():
    import concourse.bacc as bacc
    import concourse.mybir as mybir
    import concourse.tile as tile

    f32 = mybir.dt.float32
    f16 = mybir.dt.float16
    f8 = mybir.dt.float8e3
    AF = mybir.ActivationFunctionType
    MUL = mybir.AluOpType.mult
    EQ = mybir.AluOpType.is_equal

    nc = bacc.Bacc(None, target_bir_lowering=False, enable_partition_id=False)
    eaR = nc.dram_tensor("eaR", [ESH, 48], f8, kind="ExternalInput")
    xT = nc.dram_tensor("xT", [16, ESH], f16, kind="ExternalInput")
    # per-chunk sideband: cols [20b+4k+m]=sh_m, [20b+16+k]=local node id
    slq = nc.dram_tensor("slq", [CHK, 20 * NB], f16, kind="ExternalInput")
    # packed constants: w1a | w2c | R16a | R16b | R4p | iota | id16
    CW = nc.dram_tensor("CW", [128, 684], f16, kind="ExternalInput")
    ohD = nc.dram_tensor("ohD", [OH, 28 * NB], f16, kind="ExternalOutput")

    with tile.TileContext(nc) as tc:
        with (
            tc.tile_pool(name="const", bufs=1) as cp,
            tc.tile_pool(name="sb", bufs=3) as sb,
            tc.tile_pool(name="ps", bufs=1, space="PSUM") as pp,
            tc.tile_pool(name="ps2", bufs=1, space="PSUM") as pp2,
        ):
            w1a_sb = cp.tile([48, 48], f16)
            nc.sync.dma_start(out=w1a_sb[:], in_=CW[0:48, 0:48])
            w2c_sb = cp.tile([48, 320], f16)
            nc.sync.dma_start(out=w2c_sb[:], in_=CW[0:48, 48:368])
            R16a_sb = cp.tile([128, 20], f16)
            nc.sync.dma_start(out=R16a_sb[:], in_=CW[:, 368:388])
            R16b_sb = cp.tile([128, 20], f16)
            nc.sync.dma_start(out=R16b_sb[:], in_=CW[:, 388:408])
            R4p_sb = cp.tile([64, 20], f16)
            nc.sync.dma_start(out=R4p_sb[:], in_=CW[0:64, 408:428])
            iota_sb = cp.tile([CHK, OH], f16)
            nc.sync.dma_start(out=iota_sb[:], in_=CW[0:CHK, 428:428 + OH])
            id16_sb = cp.tile([CHK, CHK], f16)
            nc.sync.dma_start(out=id16_sb[:], in_=CW[0:CHK, 556:556 + CHK])

            for b in range(NB):
                s = slice(BLK * b, BLK * (b + 1))
                # --- MLP layer 1: h = relu(0.5 * w1^T ea2), fp8 input ---
                # edge-major fp8 rows in, transposed on the PE
                ea_n = sb.tile([CHK, 4 * 48], f8, tag="ea")
                for k in range(4):
                    e0 = BLK * b + CHK * k
                    nc.sync.dma_start(out=ea_n[:, 48 * k:48 * (k + 1)],
                                      in_=eaR[e0:e0 + CHK, :])
                ea_n16 = sb.tile([CHK, 4 * 48], f16, tag="ean16")
                nc.vector.tensor_copy(out=ea_n16[:, :], in_=ea_n[:, :])
                eaT_ps = pp.tile([48, 512], f16, tag="eaT")
                ea16 = sb.tile([48, BLK], f16, tag="ea16")
                for k in range(4):
                    nc.tensor.transpose(eaT_ps[:, 128 * k:128 * k + CHK],
                                        ea_n16[:, 48 * k:48 * (k + 1)],
                                        id16_sb[:])
                    nc.scalar.activation(ea16[:, CHK * k:CHK * (k + 1)],
                                         eaT_ps[:, 128 * k:128 * k + CHK],
                                         AF.Copy)
                ph = pp.tile([48, BLK], f32, tag="ph")
                nc.tensor.matmul(ph[:, :], lhsT=w1a_sb[:], rhs=ea16[:, :],
                                 start=True, stop=True)
                h_sb = sb.tile([48, BLK], f16, tag="h")
                nc.scalar.activation(h_sb[:, :], ph[:, :], AF.Relu, scale=0.5)
                # --- MLP layer 2: per-edge TP weights (permuted layout) ---
                pc1 = pp2.tile([128, BLK], f32, tag="pc1")
                pc2 = pp2.tile([128, BLK], f32, tag="pc2")
                pc3 = pp2.tile([64, BLK], f32, tag="pc3")
                nc.tensor.matmul(pc1[:, :], lhsT=w2c_sb[:, 0:128],
                                 rhs=h_sb[:, :], start=True, stop=True)
                nc.tensor.matmul(pc2[:, :], lhsT=w2c_sb[:, 128:256],
                                 rhs=h_sb[:, :], start=True, stop=True)
                nc.tensor.matmul(pc3[:, :], lhsT=w2c_sb[:, 256:320],
                                 rhs=h_sb[:, :], start=True, stop=True)
                # --- x replicated 8x across partitions ---
                xr = sb.tile([128, BLK], f16, tag="xr")
                for r in range(8):
                    nc.sync.dma_start(out=xr[16 * r:16 * (r + 1), :],
                                      in_=xT[:, s])
                # --- TP elementwise on DVE ---
                C1 = sb.tile([128, BLK], f16, tag="C1")
                C2 = sb.tile([128, BLK], f16, tag="C2")
                C3 = sb.tile([64, BLK], f16, tag="C3")
                nc.vector.tensor_tensor(out=C1[:, :], in0=xr[:, :],
                                        in1=pc1[:, :], op=MUL)
                nc.vector.tensor_tensor(out=C2[:, :], in0=xr[:, :],
                                        in1=pc2[:, :], op=MUL)
                nc.vector.tensor_tensor(out=C3[:, :], in0=xr[0:64, :],
                                        in1=pc3[:, :], op=MUL)
                # --- sideband: sh coefficients + local node ids ---
                sl16 = sb.tile([CHK, 20], f16, tag="sl16")
                nc.sync.dma_start(out=sl16[:, :], in_=slq[:, 20 * b:20 * b + 20])
                sl_sb = sb.tile([CHK, 20], f32, tag="sl")
                nc.scalar.activation(sl_sb[:, :], sl16[:, :], AF.Copy)
                # --- per chunk: edge-major reduction, sh scale, one-hot ---
                po = pp.tile([CHK, 80], f32, tag="po")
                oh = pp.tile([OH, 28], f32, tag="oh")
                for k in range(4):
                    ck = slice(CHK * k, CHK * (k + 1))
                    pk = po[:, 20 * k:20 * k + 20]
                    nc.tensor.matmul(pk, lhsT=C1[:, ck], rhs=R16a_sb[:],
                                     start=True, stop=False)
                    nc.tensor.matmul(pk, lhsT=C2[:, ck], rhs=R16b_sb[:],
                                     start=False, stop=False)
                    nc.tensor.matmul(pk, lhsT=C3[:, ck], rhs=R4p_sb[:],
                                     start=False, stop=True)
                    tpT = sb.tile([CHK, 28], f16, tag=f"tp{k}")
                    nc.vector.tensor_scalar(
                        out=tpT[:, 0:16], in0=po[:, 20 * k:20 * k + 16],
                        scalar1=sl_sb[:, 4 * k:4 * k + 1], scalar2=None, op0=MUL)
                    for m in range(3):
                        nc.vector.tensor_scalar(
                            out=tpT[:, 16 + m:28:3],
                            in0=po[:, 20 * k + 16:20 * k + 20],
                            scalar1=sl_sb[:, 4 * k + 1 + m:4 * k + 2 + m],
                            scalar2=None, op0=MUL)
                    S = sb.tile([CHK, OH], f16, tag=f"S{k}")
                    nc.vector.tensor_scalar(
                        out=S[:, :], in0=iota_sb[:, :],
                        scalar1=sl_sb[:, 16 + k:17 + k], scalar2=None, op0=EQ)
                    nc.tensor.matmul(oh[:, :], lhsT=S[:, :], rhs=tpT[:, :],
                                     start=(k == 0), stop=(k == 3))
                oh_sb = sb.tile([OH, 28], f16, tag="oh_sb")
                nc.scalar.activation(oh_sb[:, :], oh[:, :], AF.Copy)
                nc.sync.dma_start(out=ohD[:, 28 * b:28 * b + 28], in_=oh_sb[:, :])
    nc.finalize()
    return nc


def _static_weights(w1, w2):
    """Packed constant tensor CW [128, 556] f16 shared by all cores."""
    inv = np.float32(1.0 / np.sqrt(16.0))
    CW = np.zeros((128, 684), np.float16)
    CW[0:48, 0:48] = np.asarray(w1, np.float32).astype(np.float16)
    wb = np.asarray(w2, np.float32) * inv                         # [48,320]
    p = np.arange(256)
    perm0 = (p % 16) * 16 + p // 16          # pc row 16j+i <- w col i*16+j
    p = np.arange(64)
    perm1 = 256 + (p % 16) * 4 + p // 16     # pc row 16u+i <- w col 256+i*4+u
    CW[0:48, 48:368] = wb[:, np.concatenate([perm0, perm1])].astype(np.float16)
    CW[np.arange(128), 368 + np.arange(128) // 16] = 1.0          # R16a
    CW[np.arange(128), 388 + 8 + np.arange(128) // 16] = 1.0      # R16b
    CW[np.arange(64), 408 + 16 + np.arange(64) // 16] = 1.0       # R4p
    CW[0:CHK, 428:428 + OH] = np.arange(OH, dtype=np.float16)     # iota rows
    CW[0:CHK, 556:556 + CHK] = np.eye(CHK, dtype=np.float16)      # id16
    return CW


def _f8_lut():
    """uint8 table: fp16 bits -> e3m4 bits of (2 * value)."""
    if "f8lut" not in _CACHE:
        with np.errstate(invalid="ignore", over="ignore"):
            vals = np.arange(65536, dtype=np.uint16).view(np.float16)
            _CACHE["f8lut"] = (vals.astype(np.float32) * EA_SCALE).astype(F8) \
                                  .view(np.uint8)
    return _CACHE["f8lut"]


def _prep_jit():
    if "prep_jit" not in _CACHE:
        import jax
        import jax.numpy as jnp

        def f(ea, lut, perm, na, dstp, sh4):
            a16 = ea.astype(jnp.float16)
            bits = jax.lax.bitcast_convert_type(a16, jnp.uint16)
            q8 = jnp.take(lut, bits, axis=0)                      # [E,48] u8
            ea8 = jnp.take(q8, perm, axis=0)                      # [E,48] u8
            x16 = jnp.take(na, dstp, axis=0).astype(jnp.float16).T  # [16,E]
            s16 = jnp.take(sh4, perm, axis=0).astype(jnp.float16)   # [E,4]
            return ea8, x16, s16
        _CACHE["prep_jit"] = jax.jit(f)
    return _CACHE["prep_jit"]


def _install_cached_runner(nc):
    """Reuse one compiled jit across calls for our fixed-shape nc.

    run_bass_via_pjrt rebuilds jax.jit(shard_map(...)) on every invocation
    (re-trace + re-lower + compile-cache deserialization, ~0.2 s/call).
    Program and shapes never change here, so build the sharded executable
    once; inputs still flow through run_bass_kernel_spmd unchanged. Falls
    back to the stock implementation for any other nc.
    """
    if _CACHE.get("runner_installed"):
        return
    import jax
    from concourse import bass2jax
    import concourse.mybir as mybir

    orig = bass2jax.run_bass_via_pjrt
    state = {}

    def _build(n_cores):
        bass2jax.install_neuronx_cc_hook()
        in_names, out_names, out_avals, zero_shapes = [], [], [], []
        for alloc in nc.m.functions[0].allocations:
            if not isinstance(alloc, mybir.MemoryLocationSet):
                continue
            name = alloc.memorylocations[0].name
            if alloc.kind == "ExternalInput":
                in_names.append(name)
            elif alloc.kind == "ExternalOutput":
                out_names.append(name)
                shape = tuple(alloc.tensor_shape)
                dtype = mybir.dt.np(alloc.dtype)
                out_avals.append(jax.core.ShapedArray(shape, dtype))
                zero_shapes.append((shape, dtype))
        n_params = len(in_names)
        all_names = tuple(in_names + out_names)
        donate = tuple(range(n_params, n_params + len(out_names)))

        def _body(*args):
            outs = bass2jax._bass_exec_p.bind(
                *args,
                out_avals=tuple(out_avals),
                in_names=all_names,
                out_names=tuple(out_names),
                lowering_input_output_aliases=(),
                sim_require_finite=True,
                sim_require_nnan=True,
                nc=nc,
            )
            return tuple(outs)

        devices = jax.devices()[:n_cores]
        mesh = bass2jax.Mesh(np.asarray(devices), ("core",))
        nio = n_params + len(out_names)
        sharded = jax.jit(
            bass2jax.shard_map(
                _body, mesh=mesh,
                in_specs=(bass2jax.PartitionSpec("core"),) * nio,
                out_specs=(bass2jax.PartitionSpec("core"),) * len(out_names),
                check_rep=False),
            donate_argnums=donate, keep_unused=True)
        return sharded, in_names, out_names, out_avals, zero_shapes

    def patched(nc_arg, in_maps, n_cores):
        if nc_arg is not nc or n_cores != NCORES:
            return orig(nc_arg, in_maps, n_cores)
        if "r" not in state:
            try:
                state["r"] = _build(n_cores)
            except Exception:
                return orig(nc_arg, in_maps, n_cores)
        sharded, in_names, out_names, out_avals, zero_shapes = state["r"]
        concat_in = [
            np.concatenate([np.asarray(m[name]) for m in in_maps], axis=0)
            for name in in_names]
        concat_zeros = [
            np.zeros((n_cores * s[0], *s[1:]), d) for s, d in zero_shapes]
        out_arrs = sharded(*concat_in, *concat_zeros)
        return [
            {name: np.asarray(out_arrs[i]).reshape(n_cores, *out_avals[i].shape)[c]
             for i, name in enumerate(out_names)}
            for c in range(n_cores)]

    bass2jax.run_bass_via_pjrt = patched
    _CACHE["runner_installed"] = True


def kernel(node_attr, edge_index, edge_attr, edge_sh, w1, b1, w2, b2):
    global LAST_RESULTS
    import jax
    from concourse.bass_utils import run_bass_kernel_spmd

    if "jaxcfg" not in _CACHE:
        try:
            jax.config.update(
                "jax_compilation_cache_dir",
                os.path.join(tempfile.gettempdir(), "jax_cc_cache"))
            jax.config.update("jax_persistent_cache_min_compile_time_secs", 0.5)
            jax.config.update("jax_persistent_cache_min_entry_size_bytes", -1)
        except Exception:
            pass
        _CACHE["jaxcfg"] = True
    cpu = jax.devices("cpu")[0]

    src = np.asarray(edge_index[0], dtype=np.intp)
    dst = np.asarray(edge_index[1], dtype=np.intp)
    edge_attr = np.asarray(edge_attr, dtype=np.float32)
    node_attr = np.asarray(node_attr, dtype=np.float32)
    edge_sh = np.asarray(edge_sh, dtype=np.float32)
    assert not np.any(np.asarray(b1)), "nonzero b1 unsupported on device"
    assert not np.any(np.asarray(b2)), "nonzero b2 unsupported on device"

    # --- host prep: sort edges by source node, build wire tensors ---
    perm = np.argsort(src, kind="stable")
    src_s = src[perm]
    dstp = dst[perm]
    NBLK = E_TOT // BLK
    bases = np.ascontiguousarray(src_s[::BLK])               # [NBLK]
    lid = src_s - np.repeat(bases, BLK)
    assert lid.max() < OH, "block node span exceeds one-hot width"
    counts = np.bincount(src_s, minlength=N_NODES).astype(np.float32)

    sh4 = np.ascontiguousarray(edge_sh[:, 0:4])
    with jax.default_device(cpu):
        rs = _prep_jit()(edge_attr, _f8_lut(), perm, node_attr, dstp, sh4)
        jax.block_until_ready(rs)
        try:
            ea8_all, xT_all, s16 = (np.from_dlpack(r) for r in rs)
        except Exception:
            ea8_all, xT_all, s16 = (np.asarray(r) for r in rs)
    ea8_all = ea8_all.view(F8)

    # sideband: per chunk (125 edges) sh0..sh3 columns + local node id
    slq_all = np.empty((CHK, NBLK, 20), np.float16)
    slq_all[:, :, 0:16] = (
        s16.reshape(NBLK, 4, CHK, 4).transpose(2, 0, 1, 3).reshape(CHK, NBLK, 16))
    slq_all[:, :, 16:20] = (
        lid.astype(np.float16).reshape(NBLK, 4, CHK).transpose(2, 0, 1))
    slq_all = slq_all.reshape(CHK, NBLK * 20)

    CW = _static_weights(w1, w2)
    in_maps = []
    for c in range(NCORES):
        sl = slice(c * ESH, (c + 1) * ESH)
        in_maps.append({"eaR": ea8_all[sl], "xT": xT_all[:, sl],
                        "slq": slq_all[:, c * NB * 20:(c + 1) * NB * 20],
                        "CW": CW})

    if "nc" not in _CACHE:
        nc = _build_bass()
        raw = nc.to_json_bytes()       # immutable after finalize; serialize once
        nc.to_json_bytes = lambda: raw
        _CACHE["nc"] = nc
    nc = _CACHE["nc"]
    _install_cached_runner(nc)

    res = run_bass_kernel_spmd(nc, in_maps, core_ids=list(range(NCORES)))
    LAST_RESULTS = res

    # --- host post: overlap-add per-block node partials, divide by count ---
    sums = np.zeros((N_NODES + OH, 28), np.float32)
    for c in range(NCORES):
        O = np.ascontiguousarray(
            res.results[c]["ohD"].reshape(OH, NB, 28).transpose(1, 0, 2),
            dtype=np.float32)
        cb = bases[c * NB:(c + 1) * NB]
        for b in range(NB):
            sums[cb[b]:cb[b] + OH] += O[b]
    out = sums[0:N_NODES] / np.maximum(counts, 1.0)[:, None]
    return np.ascontiguousarray(out, dtype=np.float32)


# revision 21
# speedup vs baseline: 1.0879x; 1.0879x over previous
"""TensorProductConvLayer (DiffDock) Bass kernel for 8 Trainium2 cores.

The wall clock is dominated by the axon tunnel (~50-80 MB/s shared), so the
design minimizes wire bytes in both directions:
  - Edges are pre-sorted by source node on the host; each core gets a
    contiguous shard of 125K sorted edges.
  - edge_attr ships fp8 e3m4 (x2 scale, un-done by the activation scale),
    x = node_attr[dst] ships fp16 non-replicated, sh coefficients and
    block-local node ids ship as a small fp16 sideband tensor, and all
    model constants ship as one packed fp16 tensor (one device_put).
  - The device computes the per-edge MLP + tensor product AND the segment
    sum: per 500-edge block it emits [64 local nodes, 28] partial sums
    (one-hot matmul against iota-built masks), so D2H is 7 MB instead of
    per-edge outputs (40+ MB). The host overlap-adds block partials and
    divides by counts.
Per block: edge-major fp8 rows transposed on the PE (f16 identity matmul),
MLP on PE (fp16 weights), TP contraction as DVE elementwise multiply + PE
reduction emitting edge-major poT [125, 20] (operand-swapped matmul), DVE
per-partition sh scaling, is_equal one-hot build, segment matmul. Host prep
(fp16 cast -> e3m4 LUT -> sort-order gather) runs as one fused jax-CPU jit;
the serialized BIR is cached on the nc object so warm calls skip
re-serialization.
"""

import os
import tempfile
import numpy as np
import ml_dtypes

E_TOT = 1_000_000
N_NODES = 100_000
NCORES = 8
ESH = E_TOT // NCORES          # 125000 edges per core
BLK = 500
NB = ESH // BLK                # 250 blocks, no padding
CHK = 125                      # edges per transposed reduction chunk
OH = 64                        # one-hot width (max block node span, asserted)
EA_SCALE = np.float32(2.0)     # edge_attr pre-scale for fp8 e3m4 range use

F8 = ml_dtypes.float8_e3m4

_CACHE = {}
LAST_RESULTS = None


def _build_bass():
    import concourse.bacc as bacc
    import concourse.mybir as mybir
    import concourse.tile as tile

    f32 = mybir.dt.float32
    f16 = mybir.dt.float16
    f8 = mybir.dt.float8e3
    AF = mybir.ActivationFunctionType
    MUL = mybir.AluOpType.mult
    EQ = mybir.AluOpType.is_equal

    nc = bacc.Bacc(None, target_bir_lowering=False, enable_partition_id=False)
    eaR = nc.dram_tensor("eaR", [ESH, 48], f8, kind="ExternalInput")
    xT = nc.dram_tensor("xT", [16, ESH], f16, kind="ExternalInput")
    # per-chunk sideband: cols [20b+4k+m]=sh_m, [20b+16+k]=local node id
    slq = nc.dram_tensor("slq", [CHK, 20 * NB], f16, kind="ExternalInput")
    # packed constants: w1a | w2c | R16a | R16b | R4p | iota | id16
    CW = nc.dram_tensor("CW", [128, 684], f16, kind="ExternalInput")
    ohD = nc.dram_tensor("ohD", [OH, 28 * NB], f16, kind="ExternalOutput")

    with tile.TileContext(nc) as tc:
        with (
            tc.tile_pool(name="const", bufs=1) as cp,
            tc.tile_pool(name="sb", bufs=3) as sb,
            tc.tile_pool(name="ps", bufs=1, space="PSUM") as pp,
            tc.tile_pool(name="ps2", bufs=1, space="PSUM") as pp2,
        ):
            w1a_sb = cp.tile([48, 48], f16)
            nc.sync.dma_start(out=w1a_sb[:], in_=CW[0:48, 0:48])
            w2c_sb = cp.tile([48, 320], f16)
            nc.sync.dma_start(out=w2c_sb[:], in_=CW[0:48, 48:368])
            R16a_sb = cp.tile([128, 20], f16)
            nc.sync.dma_start(out=R16a_sb[:], in_=CW[:, 368:388])
            R16b_sb = cp.tile([128, 20], f16)
            nc.sync.dma_start(out=R16b_sb[:], in_=CW[:, 388:408])
            R4p_sb = cp.tile([64, 20], f16)
            nc.sync.dma_start(out=R4p_sb[:], in_=CW[0:64, 408:428])
            iota_sb = cp.tile([CHK, OH], f16)
            nc.sync.dma_start(out=iota_sb[:], in_=CW[0:CHK, 428:428 + OH])
            id16_sb = cp.tile([CHK, CHK], f16)
            nc.sync.dma_start(out=id16_sb[:], in_=CW[0:CHK, 556:556 + CHK])

            for b in range(NB):
                s = slice(BLK * b, BLK * (b + 1))
                # --- MLP layer 1: h = relu(0.5 * w1^T ea2), fp8 input ---
                # edge-major fp8 rows in, transposed on the PE
                ea_n = sb.tile([CHK, 4 * 48], f8, tag="ea")
                for k in range(4):
                    e0 = BLK * b + CHK * k
                    nc.sync.dma_start(out=ea_n[:, 48 * k:48 * (k + 1)],
                                      in_=eaR[e0:e0 + CHK, :])
                ea_n16 = sb.tile([CHK, 4 * 48], f16, tag="ean16")
                nc.vector.tensor_copy(out=ea_n16[:, :], in_=ea_n[:, :])
                eaT_ps = pp.tile([48, 512], f16, tag="eaT")
                ea16 = sb.tile([48, BLK], f16, tag="ea16")
                for k in range(4):
                    nc.tensor.transpose(eaT_ps[:, 128 * k:128 * k + CHK],
                                        ea_n16[:, 48 * k:48 * (k + 1)],
                                        id16_sb[:])
                    nc.scalar.activation(ea16[:, CHK * k:CHK * (k + 1)],
                                         eaT_ps[:, 128 * k:128 * k + CHK],
                                         AF.Copy)
                ph = pp.tile([48, BLK], f32, tag="ph")
                nc.tensor.matmul(ph[:, :], lhsT=w1a_sb[:], rhs=ea16[:, :],
                                 start=True, stop=True)
                h_sb = sb.tile([48, BLK], f16, tag="h")
                nc.scalar.activation(h_sb[:, :], ph[:, :], AF.Relu, scale=0.5)
                # --- MLP layer 2: per-edge TP weights (permuted layout) ---
                pc1 = pp2.tile([128, BLK], f32, tag="pc1")
                pc2 = pp2.tile([128, BLK], f32, tag="pc2")
                pc3 = pp2.tile([64, BLK], f32, tag="pc3")
                nc.tensor.matmul(pc1[:, :], lhsT=w2c_sb[:, 0:128],
                                 rhs=h_sb[:, :], start=True, stop=True)
                nc.tensor.matmul(pc2[:, :], lhsT=w2c_sb[:, 128:256],
                                 rhs=h_sb[:, :], start=True, stop=True)
                nc.tensor.matmul(pc3[:, :], lhsT=w2c_sb[:, 256:320],
                                 rhs=h_sb[:, :], start=True, stop=True)
                # --- x replicated 8x across partitions ---
                xr = sb.tile([128, BLK], f16, tag="xr")
                for r in range(8):
                    nc.sync.dma_start(out=xr[16 * r:16 * (r + 1), :],
                                      in_=xT[:, s])
                # --- TP elementwise on DVE ---
                C1 = sb.tile([128, BLK], f16, tag="C1")
                C2 = sb.tile([128, BLK], f16, tag="C2")
                C3 = sb.tile([64, BLK], f16, tag="C3")
                nc.vector.tensor_tensor(out=C1[:, :], in0=xr[:, :],
                                        in1=pc1[:, :], op=MUL)
                nc.vector.tensor_tensor(out=C2[:, :], in0=xr[:, :],
                                        in1=pc2[:, :], op=MUL)
                nc.vector.tensor_tensor(out=C3[:, :], in0=xr[0:64, :],
                                        in1=pc3[:, :], op=MUL)
                # --- sideband: sh coefficients + local node ids ---
                sl16 = sb.tile([CHK, 20], f16, tag="sl16")
                nc.sync.dma_start(out=sl16[:, :], in_=slq[:, 20 * b:20 * b + 20])
                sl_sb = sb.tile([CHK, 20], f32, tag="sl")
                nc.scalar.activation(sl_sb[:, :], sl16[:, :], AF.Copy)
                # --- per chunk: edge-major reduction, sh scale, one-hot ---
                po = pp.tile([CHK, 80], f32, tag="po")
                oh = pp.tile([OH, 28], f32, tag="oh")
                for k in range(4):
                    ck = slice(CHK * k, CHK * (k + 1))
                    pk = po[:, 20 * k:20 * k + 20]
                    nc.tensor.matmul(pk, lhsT=C1[:, ck], rhs=R16a_sb[:],
                                     start=True, stop=False)
                    nc.tensor.matmul(pk, lhsT=C2[:, ck], rhs=R16b_sb[:],
                                     start=False, stop=False)
                    nc.tensor.matmul(pk, lhsT=C3[:, ck], rhs=R4p_sb[:],
                                     start=False, stop=True)
                    tpT = sb.tile([CHK, 28], f16, tag=f"tp{k}")
                    nc.vector.tensor_scalar(
                        out=tpT[:, 0:16], in0=po[:, 20 * k:20 * k + 16],
                        scalar1=sl_sb[:, 4 * k:4 * k + 1], scalar2=None, op0=MUL)
                    for m in range(3):
                        nc.vector.tensor_scalar(
                            out=tpT[:, 16 + m:28:3],
                            in0=po[:, 20 * k + 16:20 * k + 20],
                            scalar1=sl_sb[:, 4 * k + 1 + m:4 * k + 2 + m],
                            scalar2=None, op0=MUL)
                    S = sb.tile([CHK, OH], f16, tag=f"S{k}")
                    nc.vector.tensor_scalar(
                        out=S[:, :], in0=iota_sb[:, :],
                        scalar1=sl_sb[:, 16 + k:17 + k], scalar2=None, op0=EQ)
                    nc.tensor.matmul(oh[:, :], lhsT=S[:, :], rhs=tpT[:, :],
                                     start=(k == 0), stop=(k == 3))
                oh_sb = sb.tile([OH, 28], f16, tag="oh_sb")
                nc.scalar.activation(oh_sb[:, :], oh[:, :], AF.Copy)
                nc.sync.dma_start(out=ohD[:, 28 * b:28 * b + 28], in_=oh_sb[:, :])
    nc.finalize()
    return nc


def _static_weights(w1, w2):
    """Packed constant tensor CW [128, 556] f16 shared by all cores."""
    inv = np.float32(1.0 / np.sqrt(16.0))
    CW = np.zeros((128, 684), np.float16)
    CW[0:48, 0:48] = np.asarray(w1, np.float32).astype(np.float16)
    wb = np.asarray(w2, np.float32) * inv                         # [48,320]
    p = np.arange(256)
    perm0 = (p % 16) * 16 + p // 16          # pc row 16j+i <- w col i*16+j
    p = np.arange(64)
    perm1 = 256 + (p % 16) * 4 + p // 16     # pc row 16u+i <- w col 256+i*4+u
    CW[0:48, 48:368] = wb[:, np.concatenate([perm0, perm1])].astype(np.float16)
    CW[np.arange(128), 368 + np.arange(128) // 16] = 1.0          # R16a
    CW[np.arange(128), 388 + 8 + np.arange(128) // 16] = 1.0      # R16b
    CW[np.arange(64), 408 + 16 + np.arange(64) // 16] = 1.0       # R4p
    CW[0:CHK, 428:428 + OH] = np.arange(OH, dtype=np.float16)     # iota rows
    CW[0:CHK, 556:556 + CHK] = np.eye(CHK, dtype=np.float16)      # id16
    return CW


def _f8_lut():
    """uint8 table: fp16 bits -> e3m4 bits of (2 * value)."""
    if "f8lut" not in _CACHE:
        with np.errstate(invalid="ignore", over="ignore"):
            vals = np.arange(65536, dtype=np.uint16).view(np.float16)
            _CACHE["f8lut"] = (vals.astype(np.float32) * EA_SCALE).astype(F8) \
                                  .view(np.uint8)
    return _CACHE["f8lut"]


def _prep_jit():
    if "prep_jit" not in _CACHE:
        import jax
        import jax.numpy as jnp

        def f(ea, lut, perm, na, dstp, sh4):
            a16 = ea.astype(jnp.float16)
            bits = jax.lax.bitcast_convert_type(a16, jnp.uint16)
            q8 = jnp.take(lut, bits, axis=0)                      # [E,48] u8
            ea8 = jnp.take(q8, perm, axis=0)                      # [E,48] u8
            x16 = jnp.take(na, dstp, axis=0).astype(jnp.float16).T  # [16,E]
            s16 = jnp.take(sh4, perm, axis=0).astype(jnp.float16)   # [E,4]
            return ea8, x16, s16
        _CACHE["prep_jit"] = jax.jit(f)
    return _CACHE["prep_jit"]


def _install_cached_runner(nc):
    """Reuse one compiled jit across calls for our fixed-shape nc.

    run_bass_via_pjrt rebuilds jax.jit(shard_map(...)) on every invocation
    (re-trace + re-lower + compile-cache deserialization, ~0.2 s/call).
    Program and shapes never change here, so build the sharded executable
    once; inputs still flow through run_bass_kernel_spmd unchanged. Falls
    back to the stock implementation for any other nc.
    """
    if _CACHE.get("runner_installed"):
        return
    import jax
    from concourse import bass2jax
    import concourse.mybir as mybir

    orig = bass2jax.run_bass_via_pjrt
    state = {}

    def _build(n_cores):
        bass2jax.install_neuronx_cc_hook()
        in_names, out_names, out_avals, zero_shapes = [], [], [], []
        for alloc in nc.m.functions[0].allocations:
            if not isinstance(alloc, mybir.MemoryLocationSet):
                continue
            name = alloc.memorylocations[0].name
            if alloc.kind == "ExternalInput":
                in_names.append(name)
            elif alloc.kind == "ExternalOutput":
                out_names.append(name)
                shape = tuple(alloc.tensor_shape)
                dtype = mybir.dt.np(alloc.dtype)
                out_avals.append(jax.core.ShapedArray(shape, dtype))
                zero_shapes.append((shape, dtype))
        n_params = len(in_names)
        all_names = tuple(in_names + out_names)
        donate = tuple(range(n_params, n_params + len(out_names)))

        def _body(*args):
            outs = bass2jax._bass_exec_p.bind(
                *args,
                out_avals=tuple(out_avals),
                in_names=all_names,
                out_names=tuple(out_names),
                lowering_input_output_aliases=(),
                sim_require_finite=True,
                sim_require_nnan=True,
                nc=nc,
            )
            return tuple(outs)

        devices = jax.devices()[:n_cores]
        mesh = bass2jax.Mesh(np.asarray(devices), ("core",))
        nio = n_params + len(out_names)
        sharded = jax.jit(
            bass2jax.shard_map(
                _body, mesh=mesh,
                in_specs=(bass2jax.PartitionSpec("core"),) * nio,
                out_specs=(bass2jax.PartitionSpec("core"),) * len(out_names),
                check_rep=False),
            donate_argnums=donate, keep_unused=True)
        return sharded, in_names, out_names, out_avals, zero_shapes

    def patched(nc_arg, in_maps, n_cores):
        if nc_arg is not nc or n_cores != NCORES:
            return orig(nc_arg, in_maps, n_cores)
        if "r" not in state:
            try:
                state["r"] = _build(n_cores)
            except Exception:
                return orig(nc_arg, in_maps, n_cores)
        sharded, in_names, out_names, out_avals, zero_shapes = state["r"]
        concat_in = [
            np.concatenate([np.asarray(m[name]) for m in in_maps], axis=0)
            for name in in_names]
        concat_zeros = [
            np.zeros((n_cores * s[0], *s[1:]), d) for s, d in zero_shapes]
        out_arrs = sharded(*concat_in, *concat_zeros)
        try:
            # fetch the 8 per-core shards concurrently: the 7 MB output is
            # latency-bound (8 serial ~40 ms round trips), threads hide it
            import concurrent.futures as cf
            fetched = []
            for i in range(len(out_names)):
                shards = sorted(out_arrs[i].addressable_shards,
                                key=lambda s: s.index[0].start or 0)
                assert len(shards) == n_cores
                with cf.ThreadPoolExecutor(n_cores) as ex:
                    fetched.append(list(ex.map(
                        lambda s: np.asarray(s.data), shards)))
            return [
                {name: fetched[i][c] for i, name in enumerate(out_names)}
                for c in range(n_cores)]
        except Exception:
            return [
                {name: np.asarray(out_arrs[i]).reshape(
                    n_cores, *out_avals[i].shape)[c]
                 for i, name in enumerate(out_names)}
                for c in range(n_cores)]

    bass2jax.run_bass_via_pjrt = patched
    _CACHE["runner_installed"] = True


def kernel(node_attr, edge_index, edge_attr, edge_sh, w1, b1, w2, b2):
    global LAST_RESULTS
    import jax
    from concourse.bass_utils import run_bass_kernel_spmd

    if "jaxcfg" not in _CACHE:
        try:
            jax.config.update(
                "jax_compilation_cache_dir",
                os.path.join(tempfile.gettempdir(), "jax_cc_cache"))
            jax.config.update("jax_persistent_cache_min_compile_time_secs", 0.5)
            jax.config.update("jax_persistent_cache_min_entry_size_bytes", -1)
        except Exception:
            pass
        _CACHE["jaxcfg"] = True
    cpu = jax.devices("cpu")[0]

    src = np.asarray(edge_index[0], dtype=np.intp)
    dst = np.asarray(edge_index[1], dtype=np.intp)
    edge_attr = np.asarray(edge_attr, dtype=np.float32)
    node_attr = np.asarray(node_attr, dtype=np.float32)
    edge_sh = np.asarray(edge_sh, dtype=np.float32)
    assert not np.any(np.asarray(b1)), "nonzero b1 unsupported on device"
    assert not np.any(np.asarray(b2)), "nonzero b2 unsupported on device"

    # --- host prep: sort edges by source node, build wire tensors ---
    perm = np.argsort(src, kind="stable")
    src_s = src[perm]
    dstp = dst[perm]
    NBLK = E_TOT // BLK
    bases = np.ascontiguousarray(src_s[::BLK])               # [NBLK]
    lid = src_s - np.repeat(bases, BLK)
    assert lid.max() < OH, "block node span exceeds one-hot width"
    counts = np.bincount(src_s, minlength=N_NODES).astype(np.float32)

    sh4 = np.ascontiguousarray(edge_sh[:, 0:4])
    with jax.default_device(cpu):
        rs = _prep_jit()(edge_attr, _f8_lut(), perm, node_attr, dstp, sh4)
        jax.block_until_ready(rs)
        try:
            ea8_all, xT_all, s16 = (np.from_dlpack(r) for r in rs)
        except Exception:
            ea8_all, xT_all, s16 = (np.asarray(r) for r in rs)
    ea8_all = ea8_all.view(F8)

    # sideband: per chunk (125 edges) sh0..sh3 columns + local node id
    slq_all = np.empty((CHK, NBLK, 20), np.float16)
    slq_all[:, :, 0:16] = (
        s16.reshape(NBLK, 4, CHK, 4).transpose(2, 0, 1, 3).reshape(CHK, NBLK, 16))
    slq_all[:, :, 16:20] = (
        lid.astype(np.float16).reshape(NBLK, 4, CHK).transpose(2, 0, 1))
    slq_all = slq_all.reshape(CHK, NBLK * 20)

    CW = _static_weights(w1, w2)
    in_maps = []
    for c in range(NCORES):
        sl = slice(c * ESH, (c + 1) * ESH)
        in_maps.append({"eaR": ea8_all[sl], "xT": xT_all[:, sl],
                        "slq": slq_all[:, c * NB * 20:(c + 1) * NB * 20],
                        "CW": CW})

    if "nc" not in _CACHE:
        nc = _build_bass()
        raw = nc.to_json_bytes()       # immutable after finalize; serialize once
        nc.to_json_bytes = lambda: raw
        _CACHE["nc"] = nc
    nc = _CACHE["nc"]
    _install_cached_runner(nc)

    res = run_bass_kernel_spmd(nc, in_maps, core_ids=list(range(NCORES)))
    LAST_RESULTS = res

    # --- host post: overlap-add per-block node partials, divide by count ---
    sums = np.zeros((N_NODES + OH, 28), np.float32)
    for c in range(NCORES):
        O = np.ascontiguousarray(
            res.results[c]["ohD"].reshape(OH, NB, 28).transpose(1, 0, 2),
            dtype=np.float32)
        cb = bases[c * NB:(c + 1) * NB]
        for b in range(NB):
            sums[cb[b]:cb[b] + OH] += O[b]
    out = sums[0:N_NODES] / np.maximum(counts, 1.0)[:, None]
    return np.ascontiguousarray(out, dtype=np.float32)
